# revision 1
# baseline (speedup 1.0000x reference)
"""Block-sparse attention (CABAttention) Trainium2 kernel.

Sharding: 8 cores = 2 batches x 4 head-groups (4 heads each).
Per core: qkv projection (fp32r), top-2+diag block-sparse attention
(fp16 values path, fp32 PSUM/softmax-denominator), output projection.
Block selection (top-2 of coarse block-mean scores) is computed on host
in float64 (the PE's fp32 matmul error ~1.3e-4 would flip near-tied
blocks; min top2/3rd gap in c is ~6.5e-6) and passed as index inputs,
consumed on device via dynamic access-pattern offsets on the matmul
moving operand.
Unshard: partial output projections summed over the 4 cores per batch
(row-parallel tensor split), plus bias.
"""
import sys

sys.path.insert(0, "/opt/trn_rl_repo")

import numpy as np

import concourse.bass as bass
import concourse.mybir as mybir
import concourse.tile as tile
from concourse import bacc
from concourse.bass import ds
from concourse.bass_utils import run_bass_kernel_spmd
from concourse.masks import make_identity

F32 = mybir.dt.float32
F32R = mybir.dt.float32r
F16 = mybir.dt.float16
I32 = mybir.dt.int32

DIM = 1024
H = 16
HD = 64
BS = 64
N = 2048
B = 2
M = N // BS            # 32 blocks
SCALE = HD ** -0.5
NCORES = 8
HPC = H // (NCORES // B)   # 4 heads per core

_NC_CACHE = None
LAST_RESULTS = None


def build_kernel(stage=4, sub=6, dyn='both'):
    nc = bacc.Bacc(None)
    xt_d = nc.dram_tensor("xt", [DIM, N], F32R, kind="ExternalInput")
    wq_d = nc.dram_tensor("wq", [DIM, 768], F32R, kind="ExternalInput")
    pw_d = nc.dram_tensor("pw", [256, DIM], F16, kind="ExternalInput")
    idx_d = nc.dram_tensor("selidx", [1, 256], I32, kind="ExternalInput")
    wb_d = nc.dram_tensor("wbias", [128, 64], F32, kind="ExternalInput")
    y_d = nc.dram_tensor("y", [N, DIM], F32, kind="ExternalOutput")

    with tile.TileContext(nc) as tc:
        with tc.tile_pool(name="big", bufs=1) as big, \
             tc.tile_pool(name="wrk", bufs=4) as wrk:

            # ---- persistent SBUF tensors ----
            xt = big.tile([128, 8, N], F32R)          # x^T, feature-major
            wq = big.tile([128, 8, 768], F32R)        # qkv weights^T
            pwt = big.tile([128, 2, DIM], F16)        # proj weights
            idx = big.tile([1, 256], I32)
            wb = big.tile([128, 64], F32)
            qT = [big.tile([128, N], F16, name=f"qT{i}") for i in range(2)]
            kkT = [big.tile([128, N], F16, name=f"kkT{i}") for i in range(2)]
            vvT = [big.tile([128, N], F16, name=f"vvT{i}") for i in range(2)]
            vdA = [big.tile([64, N], F16, name=f"vdA{i}") for i in range(2)]
            vdB = [big.tile([64, N], F16, name=f"vdB{i}") for i in range(2)]
            outT = [big.tile([128, N], F16, name=f"outT{i}") for i in range(2)]
            qTB = [big.tile([64, N], F16, name=f"qTB{i}") for i in range(2)]
            kkTB = [big.tile([64, N], F16, name=f"kkTB{i}") for i in range(2)]
            identf = big.tile([128, 128], F32)
            ident = big.tile([128, 128], F16)

            # ---- input DMAs (split for pipelining) ----
            xt_v = xt_d[:].rearrange("(a p) n -> p a n", p=128)
            wq_v = wq_d[:].rearrange("(a p) n -> p a n", p=128)
            pw_v = pw_d[:].rearrange("(a p) n -> p a n", p=128)
            for k in range(8):
                nc.sync.dma_start(xt[:, k, :], xt_v[:, k, :])
                nc.sync.dma_start(wq[:, k, :], wq_v[:, k, :])
            nc.sync.dma_start(pwt[:], pw_v[:])
            nc.sync.dma_start(idx[:], idx_d[:])
            nc.sync.dma_start(wb[:], wb_d[:])

            for t_ in outT:
                nc.gpsimd.memset(t_[:], 0.0)
            make_identity(nc, identf[:])
            nc.vector.tensor_copy(ident[:], identf[:])

            # ---- qkv projection: fp32r accumulate over 8 K-chunks ----
            # M-tiles: 0,1 -> qT pair0/1; 2,4 -> kkT; 3,5 -> vvT
            tgt = [qT[0], qT[1], kkT[0], vvT[0], kkT[1], vvT[1]]
            with tc.tile_pool(name="qkps", bufs=4, space="PSUM") as qkps:
                for mt in range(6):
                    for nt in range(4):
                        ps = qkps.tile([128, 512], F32)
                        for k in range(8):
                            nc.tensor.matmul(
                                ps[:], lhsT=wq[:, k, mt * 128:(mt + 1) * 128],
                                rhs=xt[:, k, nt * 512:(nt + 1) * 512],
                                start=(k == 0), stop=(k == 7))
                        nc.vector.tensor_copy(
                            tgt[mt][:, nt * 512:(nt + 1) * 512], ps[:])

            # replicate head-B halves to partition base 0 (dynamic-offset
            # matmul operands must have partition base 0)
            for p in range(2):
                nc.sync.dma_start(qTB[p][:], qT[p][64:128, :])
                nc.sync.dma_start(kkTB[p][:], kkT[p][64:128, :])

            # ---- v_dup: per pair, transpose vvT blocks to keys-major ----
            if stage < 2:
                vdup_pairs = []
            elif True:
                vdup_pairs = [0, 1]
            # vd[p][:, j*64:(j+1)*64]        = v-block j, head A (keys x hd)
            # vd[p][:, 2048 + j*64 : ...]    = v-block j, head B
            with tc.tile_pool(name="vtps", bufs=2, space="PSUM") as vtps:
                for p in vdup_pairs:
                    for j in range(M):
                        tp = vtps.tile([64, 128], F16)
                        nc.tensor.transpose(
                            tp[:], vvT[p][:, j * 64:(j + 1) * 64], ident[:])
                        nc.vector.tensor_copy(
                            vdA[p][:, j * 64:(j + 1) * 64], tp[:, 0:64])
                        nc.vector.tensor_copy(
                            vdB[p][:, j * 64:(j + 1) * 64], tp[:, 64:128])

            # ---- block-sparse attention ----
            with tc.tile_pool(name="spsp", bufs=2, space="PSUM") as spsp, \
                 tc.tile_pool(name="ptps", bufs=2, space="PSUM") as ptps, \
                 tc.tile_pool(name="avps", bufs=2, space="PSUM") as avps, \
                 tc.tile_pool(name="otps", bufs=2, space="PSUM") as otps:
                anchors = []
                for p in (range(2) if stage >= 3 else []):
                    for qb in range(M):
                        t = p * M + qb
                        base = p * 128 + qb * 4
                        offs = []
                        if stage >= 4:
                            for c in range(4):
                                eng = nc.tensor
                                tmp = eng.alloc_register(f"off_{base + c}")
                                li = eng.reg_load(
                                    tmp, idx[0:1, base + c:base + c + 1])
                                if t >= 3:
                                    tile.add_dep_helper(
                                        li.ins, anchors[t - 3].ins, sync=False,
                                        reason="bound PE register live range")
                                offs.append(eng.snap(tmp, donate=True,
                                                     min_val=0,
                                                     max_val=N - 64))
                        else:
                            offs = [qb * 64] * 4
                        qs = slice(qb * 64, (qb + 1) * 64)
                        sps = spsp.tile([128, 192], F32)
                        SUB = sub
                        # scores: slots s0,s1 dynamic; s2 = diagonal (static)
                        so = offs if dyn in ('both', 'scores') else [qb * 64] * 4
                        rhs_off = [so[0], so[1], qb * 64]
                        rhs_off_b = [so[2], so[3], qb * 64]
                        mi0 = None
                        for s in range(3):
                            cs = slice(s * 64, (s + 1) * 64)
                            nc.tensor.matmul(
                                sps[0:64, cs], lhsT=qT[p][0:64, qs],
                                rhs=kkT[p][0:64, ds(rhs_off[s], 64)],
                                start=True, stop=True)
                            mi0 = nc.tensor.matmul(
                                sps[64:128, cs], lhsT=qTB[p][:, qs],
                                rhs=kkTB[p][:, ds(rhs_off_b[s], 64)],
                                start=True, stop=True,
                                skip_group_check=True,
                                tile_position=(0, 64))
                        if SUB < 2:
                            continue
                        # mask duplicated diag slot (bias -30000 -> exp 0)
                        nc.vector.tensor_scalar(
                            sps[:, 128:192], sps[:, 128:192],
                            wb[:, p * 32 + qb:p * 32 + qb + 1], None,
                            op0=mybir.AluOpType.add)
                        # exp + rowsum
                        pu = wrk.tile([128, 192], F16, tag="pu")
                        den = wrk.tile([128, 1], F32, tag="den")
                        nc.scalar.activation(pu[:], sps[:],
                                             mybir.ActivationFunctionType.Exp,
                                             accum_out=den[:])
                        if SUB < 3:
                            continue
                        rden = wrk.tile([128, 1], F32, tag="rden")
                        nc.vector.reciprocal(rden[:], den[:])
                        pr = wrk.tile([128, 192], F16, tag="pr")
                        nc.vector.tensor_scalar(pr[:], pu[:], rden[:, 0:1],
                                                None,
                                                op0=mybir.AluOpType.mult)
                        if SUB < 4:
                            continue
                        # transpose probs per slot: [128q,64k] -> [64k,128q]
                        pt = ptps.tile([64, 384], F16)
                        for s in range(3):
                            nc.tensor.transpose(
                                pt[:, s * 128:(s + 1) * 128],
                                pr[:, s * 64:(s + 1) * 64], ident[:])
                        pts = wrk.tile([64, 384], F16, tag="pts")
                        nc.vector.tensor_copy(pts[:], pt[:])
                        if SUB < 5:
                            anchors.append(mi0)
                            continue
                        # AV: out[q, hd] accumulated over slots (dynamic rhs)
                        avpAB = avps.tile([64, 128], F32)
                        avpA = avpAB[:, 0:64]
                        avpB = avpAB[:, 64:128]
                        ao = offs if dyn in ('both', 'av') else [qb * 64] * 4
                        av_off = [ao[0], ao[1], qb * 64]
                        av_off_b = [ao[2], ao[3], qb * 64]
                        for s in range(3):
                            nc.tensor.matmul(
                                avpA,
                                lhsT=pts[:, s * 128:s * 128 + 64],
                                rhs=vdA[p][:, ds(av_off[s], 64)],
                                start=(s == 0), stop=(s == 2))
                        for s in range(3):
                            mi = nc.tensor.matmul(
                                avpB,
                                lhsT=pts[:, s * 128 + 64:s * 128 + 128],
                                rhs=vdB[p][:, ds(av_off_b[s], 64)],
                                start=(s == 0), stop=(s == 2))
                        if SUB >= 5:
                            anchors.append(mi)
                        if SUB < 6:
                            continue
                        av_sb = wrk.tile([64, 128], F16, tag="av_sb")
                        nc.scalar.copy(av_sb[:], avpAB[:])
                        # transpose back to [hd, q] for the projection
                        otp = otps.tile([128, 64], F16)
                        nc.tensor.transpose(otp[:], av_sb[:], ident[0:64, 0:64])
                        nc.vector.tensor_copy(outT[p][:, qs], otp[:])

            # ---- output projection: y = outT.T @ pw ----
            with tc.tile_pool(name="ypsp", bufs=4, space="PSUM") as ypsp:
                for tt in range(16):
                    ts_ = slice(tt * 128, (tt + 1) * 128)
                    for nt in range(2):
                        ns = slice(nt * 512, (nt + 1) * 512)
                        yp = ypsp.tile([128, 512], F32)
                        nc.tensor.matmul(yp[:], lhsT=outT[0][:, ts_],
                                         rhs=pwt[:, 0, ns],
                                         start=True, stop=False)
                        nc.tensor.matmul(yp[:], lhsT=outT[1][:, ts_],
                                         rhs=pwt[:, 1, ns],
                                         start=False, stop=True)
                        ys = wrk.tile([128, 512], F32, tag="ys")
                        nc.vector.tensor_copy(ys[:], yp[:])
                        nc.sync.dma_start(y_d[ts_, ns], ys[:])

    nc.finalize()
    return nc


def _host_prep(x, qkv_w, proj_w):
    """Per-core input maps + block selection (float64, matches fp32 ref)."""
    in_maps = []
    x64 = x.astype(np.float64)
    for core in range(NCORES):
        b = core // (NCORES // B)
        hg = core % (NCORES // B)
        heads = [hg * HPC + i for i in range(HPC)]

        xt = np.ascontiguousarray(x[b].T).astype(np.float32)

        wqkvT = np.empty((DIM, 768), np.float32)
        for p in range(2):
            hA, hB = heads[2 * p], heads[2 * p + 1]
            wqkvT[:, p*128:p*128+64] = qkv_w[hA*64:(hA+1)*64].T * SCALE
            wqkvT[:, p*128+64:p*128+128] = qkv_w[hB*64:(hB+1)*64].T * SCALE
            kbase = 256 + p * 256
            wqkvT[:, kbase:kbase+64] = qkv_w[DIM+hA*64:DIM+(hA+1)*64].T
            wqkvT[:, kbase+64:kbase+128] = qkv_w[DIM+hB*64:DIM+(hB+1)*64].T
            vbase = kbase + 128
            wqkvT[:, vbase:vbase+64] = qkv_w[2*DIM+hA*64:2*DIM+(hA+1)*64].T
            wqkvT[:, vbase+64:vbase+128] = qkv_w[2*DIM+hB*64:2*DIM+(hB+1)*64].T

        pw = np.ascontiguousarray(
            proj_w[:, heads[0]*64:(heads[-1]+1)*64].T).astype(np.float16)

        # float64 selection (matches fp32 reference ordering w/ margin)
        xb = x64[b].reshape(M, BS, DIM).mean(axis=1)
        selidx = np.zeros((1, 256), np.int32)
        wbias = np.zeros((128, 64), np.float32)
        for p in range(2):
            for hip in range(2):
                h = heads[2 * p + hip]
                qb_ = xb @ qkv_w[h*64:(h+1)*64].T.astype(np.float64)
                kb_ = xb @ qkv_w[DIM+h*64:DIM+(h+1)*64].T.astype(np.float64)
                c = qb_ @ kb_.T
                for i in range(M):
                    order = np.argsort(-c[i], kind="stable")
                    i1, i2 = int(order[0]), int(order[1])
                    col = p * 128 + i * 4 + hip * 2
                    selidx[0, col] = i1 * 64
                    selidx[0, col + 1] = i2 * 64
                    if i == i1 or i == i2:
                        wbias[hip*64:(hip+1)*64, p*32+i] = -30000.0
        in_maps.append({"xt": xt, "wq": wqkvT, "pw": pw,
                        "selidx": selidx, "wbias": wbias})
    return in_maps


def kernel(x, qkv_w, proj_w, proj_b):
    global _NC_CACHE, LAST_RESULTS
    x = np.asarray(x, np.float32)
    qkv_w = np.asarray(qkv_w, np.float32)
    proj_w = np.asarray(proj_w, np.float32)
    proj_b = np.asarray(proj_b, np.float32)

    if _NC_CACHE is None:
        _NC_CACHE = build_kernel()
    nc = _NC_CACHE

    in_maps = _host_prep(x, qkv_w, proj_w)
    res = run_bass_kernel_spmd(nc, in_maps, list(range(NCORES)))
    LAST_RESULTS = res

    out = np.zeros((B, N, DIM), np.float32)
    for core in range(NCORES):
        out[core // (NCORES // B)] += res.results[core]["y"]
    out += proj_b[None, None, :]
    return out



# revision 11
# speedup vs baseline: 1.3039x; 1.3039x over previous
"""Block-sparse attention (CABAttention) Trainium2 kernel, v2.

Sharding: 8 cores = 2 batches x 4 head-groups (2 pairs of heads each).
Per core, all in fp16 (fp32 PSUM/denominators):
 - qkv projection: fp16 matmuls over 8 K-chunks
 - top-2+diag block-sparse attention; block selection on host in
   float64 (matches fp32 reference ordering), consumed as dynamic
   access-pattern offsets on matmul moving operands (4 offsets per
   block loaded with one multi-register PE load)
 - softmax denominator via activation accum_out; normalization applied
   AFTER the AV matmul on [128,64] outputs (cheaper than normalizing
   [128,192] probabilities)
 - per-slot [64,64] PE transposes packed into disjoint quadrants
   (A row-group 0 / B row-group 64) so pairs run concurrently
 - head-B score/AV matmuls live natively in quadrant (0,64) with
   base-0 replicas (qkB = [qT-B | kkT-B], vdAB = [vd-A | vd-B]) so a
   single offset value serves both the scores and AV dynamic APs
 - output projection row-parallel: fp16 partial y per core, summed on
   host in fp32 (+bias).
"""
import sys

sys.path.insert(0, "/opt/trn_rl_repo")

import numpy as np

import concourse.bass as bass
import concourse.mybir as mybir
import concourse.tile as tile
from concourse import bacc
from concourse.bass import ds
from concourse.bass_utils import run_bass_kernel_spmd
from concourse.masks import make_identity

F32 = mybir.dt.float32
F16 = mybir.dt.float16
I32 = mybir.dt.int32

DIM = 1024
H = 16
HD = 64
BS = 64
N = 2048
B = 2
M = N // BS            # 32 blocks
SCALE = HD ** -0.5
NCORES = 8
HPC = H // (NCORES // B)   # 4 heads per core

_NC_CACHE = None
LAST_RESULTS = None


def build_kernel(stage=5, sub=3):
    nc = bacc.Bacc(None)
    xt_d = nc.dram_tensor("xt", [DIM, N], F16, kind="ExternalInput")
    wq_d = nc.dram_tensor("wq", [DIM, 768], F16, kind="ExternalInput")
    pw_d = nc.dram_tensor("pw", [256, DIM], F16, kind="ExternalInput")
    idx_d = nc.dram_tensor("selidx", [1, 256], I32, kind="ExternalInput")
    wb_d = nc.dram_tensor("wbias", [128, 64], F32, kind="ExternalInput")
    y_d = nc.dram_tensor("y", [N, DIM], F16, kind="ExternalOutput")

    EXP = mybir.ActivationFunctionType.Exp
    ADD = mybir.AluOpType.add
    MUL = mybir.AluOpType.mult

    with tile.TileContext(nc) as tc:
        with tc.tile_pool(name="big", bufs=1) as big, \
             tc.tile_pool(name="wrk", bufs=4) as wrk:

            # ---- persistent SBUF tensors ----
            xts = big.tile([128, 8, N], F16)
            wqs = big.tile([128, 8, 768], F16)
            pwt = big.tile([128, 2, DIM], F16)
            idx = big.tile([1, 256], I32)
            wb = big.tile([128, 64], F32)
            qT = [big.tile([128, N], F16, name=f"qT{i}") for i in range(2)]
            kkT = [big.tile([128, N], F16, name=f"kkT{i}") for i in range(2)]
            vvT = [big.tile([128, N], F16, name=f"vvT{i}") for i in range(2)]
            # base-0 replicas for head B: [qT-B | kkT-B] so one dynamic
            # offset value (N + 64j) indexes the kk region
            qkB = [big.tile([64, 2 * N], F16, name=f"qkB{i}") for i in range(2)]
            # [vd-A | vd-B] keys-major v, same offset convention
            vdAB = [big.tile([64, 2 * N], F16, name=f"vdAB{i}") for i in range(2)]
            outT = [big.tile([128, N], F16, name=f"outT{i}") for i in range(2)]
            identf = big.tile([128, 128], F32)
            ident = big.tile([128, 128], F16)

            # ---- input DMAs (split for pipelining) ----
            xt_v = xt_d[:].rearrange("(a p) n -> p a n", p=128)
            wq_v = wq_d[:].rearrange("(a p) n -> p a n", p=128)
            pw_v = pw_d[:].rearrange("(a p) n -> p a n", p=128)
            for k in range(8):
                nc.sync.dma_start(xts[:, k, :], xt_v[:, k, :])
                nc.sync.dma_start(wqs[:, k, :], wq_v[:, k, :])
            nc.sync.dma_start(pwt[:], pw_v[:])
            nc.sync.dma_start(idx[:], idx_d[:])
            nc.sync.dma_start(wb[:], wb_d[:])

            make_identity(nc, identf[:])
            nc.vector.tensor_copy(ident[:], identf[:])
            if stage < 5:
                for t_ in outT:
                    nc.gpsimd.memset(t_[:], 0.0)

            def copy_out(eng, dst, src):
                if eng is nc.scalar:
                    eng.copy(dst, src)
                else:
                    eng.tensor_copy(dst, src)

            # ---- qkv projection: fp16, 8 K-chunk accumulation ----
            # mt -> target; pair-0 tiles first so attention p0 can start
            tgt = {0: qT[0], 1: qT[1], 2: kkT[0], 3: vvT[0],
                   4: kkT[1], 5: vvT[1]}
            ci = 0
            with tc.tile_pool(name="qkps", bufs=4, space="PSUM") as qkps, \
                 tc.tile_pool(name="vtps", bufs=2, space="PSUM") as vtps:

                def emit_qkv(mts):
                    nonlocal ci
                    for mt in mts:
                        for nt in range(4):
                            ps = qkps.tile([128, 512], F32)
                            for k in range(8):
                                nc.tensor.matmul(
                                    ps[:],
                                    lhsT=wqs[:, k, mt * 128:(mt + 1) * 128],
                                    rhs=xts[:, k, nt * 512:(nt + 1) * 512],
                                    start=(k == 0), stop=(k == 7))
                            eng = (nc.vector, nc.scalar)[ci % 2]
                            ci += 1
                            copy_out(eng, tgt[mt][:, nt * 512:(nt + 1) * 512],
                                     ps[:])

                def emit_vdup(p):
                    nonlocal ci
                    for j in range(M):
                        tp = vtps.tile([64, 128], F16)
                        nc.tensor.transpose(
                            tp[:], vvT[p][:, j * 64:(j + 1) * 64], ident[:])
                        # one strided copy: A block -> cols [j*64], B block
                        # -> cols [N + j*64] of vdAB
                        dst = vdAB[p][:].rearrange(
                            "q (g c) -> q g c", g=2)[:, :, j * 64:(j + 1) * 64]
                        src = tp[:].rearrange("q (g c) -> q g c", g=2)
                        eng = (nc.vector, nc.scalar)[ci % 2]
                        ci += 1
                        copy_out(eng, dst, src)

                emit_qkv((0, 2, 3))
                nc.sync.dma_start(qkB[0][:, 0:N], qT[0][64:128, :])
                nc.sync.dma_start(qkB[0][:, N:2 * N], kkT[0][64:128, :])
                emit_vdup(0)
                emit_qkv((1, 4, 5))
                nc.sync.dma_start(qkB[1][:, 0:N], qT[1][64:128, :])
                nc.sync.dma_start(qkB[1][:, N:2 * N], kkT[1][64:128, :])
                emit_vdup(1)

            # ---- block-sparse attention ----
            with tc.tile_pool(name="spsp", bufs=2, space="PSUM") as spsp, \
                 tc.tile_pool(name="ptps", bufs=2, space="PSUM") as ptps, \
                 tc.tile_pool(name="avps", bufs=2, space="PSUM") as avps, \
                 tc.tile_pool(name="otps", bufs=2, space="PSUM") as otps:
                anchors = []
                for p in (range(2) if stage >= 2 else []):
                    for qb in range(M):
                        t = p * M + qb
                        base = p * 128 + qb * 4
                        col = p * 32 + qb
                        qs = slice(qb * 64, (qb + 1) * 64)

                        # 4 dynamic offsets
                        eng = nc.tensor
                        MULTI_LOAD = True
                        offs = []
                        if MULTI_LOAD:
                            regs = [eng.alloc_register(f"off_{base + c}")
                                    for c in range(4)]
                            li = eng.reg_load(regs, idx[0:1, base:base + 4])
                            if t >= 3:
                                tile.add_dep_helper(
                                    li.ins, anchors[t - 3].ins, sync=False,
                                    reason="bound PE register live range")
                            for c in range(4):
                                lo = 0 if c < 2 else N
                                offs.append(eng.snap(
                                    regs[c], donate=True, min_val=lo,
                                    max_val=lo + N - 64))
                        else:
                            for c in range(4):
                                tmp = eng.alloc_register(f"off_{base + c}")
                                li = eng.reg_load(
                                    tmp, idx[0:1, base + c:base + c + 1])
                                if t >= 3 and c == 0:
                                    tile.add_dep_helper(
                                        li.ins, anchors[t - 3].ins, sync=False,
                                        reason="bound PE register live range")
                                lo = 0 if c < 2 else N
                                offs.append(eng.snap(
                                    tmp, donate=True, min_val=lo,
                                    max_val=lo + N - 64))
                        soA = [offs[0], offs[1], qb * 64]
                        soB = [offs[2], offs[3], N + qb * 64]

                        # scores^T quadrant-packed: A rows 0-63 via
                        # (0,0), B rows 64-127 via (0,64)
                        sps = spsp.tile([128, 192], F32)
                        for s in range(3):
                            cs = slice(s * 64, (s + 1) * 64)
                            nc.tensor.matmul(
                                sps[0:64, cs], lhsT=qT[p][0:64, qs],
                                rhs=kkT[p][0:64, ds(soA[s], 64)],
                                start=True, stop=True)
                            nc.tensor.matmul(
                                sps[64:128, cs], lhsT=qkB[p][:, qs],
                                rhs=qkB[p][:, ds(soB[s], 64)],
                                start=True, stop=True,
                                skip_group_check=True,
                                tile_position=(0, 64))

                        # exp; duplicate-diag mask via activation bias;
                        # denominators via accum_out
                        pu = wrk.tile([128, 192], F16, tag="pu")
                        den1 = wrk.tile([128, 1], F32, tag="den1")
                        den2 = wrk.tile([128, 1], F32, tag="den2")
                        den = wrk.tile([128, 1], F32, tag="den")
                        nc.scalar.activation(pu[:, 0:128], sps[:, 0:128],
                                             EXP, accum_out=den1[:])
                        nc.scalar.activation(pu[:, 128:192], sps[:, 128:192],
                                             EXP, bias=wb[:, col:col + 1],
                                             accum_out=den2[:])
                        nc.vector.tensor_tensor(den[:], den1[:], den2[:], ADD)

                        if stage < 3:
                            anchors.append(li)
                            continue
                        # transpose probs per (head, slot): [64,64] pairs
                        # in disjoint quadrants run concurrently
                        # full-height transpose per slot: [128 q, 64 k]
                        # -> [64 k, 128 (qA|qB)], both heads at base 0
                        pt = ptps.tile([64, 384], F16)
                        for s in range(3):
                            cs = slice(s * 64, (s + 1) * 64)
                            nc.tensor.transpose(
                                pt[0:64, s * 128:(s + 1) * 128],
                                pu[:, cs], ident[:])
                        pts = wrk.tile([64, 384], F16, tag="pts")
                        copy_out((nc.vector, nc.scalar)[t % 2], pts[:], pt[:])

                        if stage < 4:
                            anchors.append(li)
                            continue
                        # AV (unnormalized): A -> av[0:64], B -> av[64:128]
                        av = avps.tile([128, 64], F32)
                        for s in range(3):
                            nc.tensor.matmul(
                                av[0:64, :],
                                lhsT=pts[0:64, s * 128:s * 128 + 64],
                                rhs=vdAB[p][:, ds(soA[s], 64)],
                                start=(s == 0), stop=(s == 2))
                            mi = nc.tensor.matmul(
                                av[64:128, :],
                                lhsT=pts[0:64, s * 128 + 64:(s + 1) * 128],
                                rhs=vdAB[p][:, ds(soB[s], 64)],
                                start=(s == 0), stop=(s == 2),
                                skip_group_check=True,
                                tile_position=(0, 64))
                        anchors.append(mi)

                        # normalize post-AV (64 cols instead of 192)
                        rden = wrk.tile([128, 1], F32, tag="rden")
                        nc.vector.reciprocal(rden[:], den[:])
                        o = wrk.tile([128, 64], F16, tag="o")
                        nc.vector.tensor_scalar(o[:], av[:], rden[:, 0:1],
                                                None, op0=MUL)

                        if stage < 5:
                            continue
                        # transpose back to [hd, q]; A/B quadrant pair
                        otp = otps.tile([128, 64], F16)
                        nc.tensor.transpose(otp[0:64, :], o[0:64, :],
                                            ident[0:64, 0:64])
                        nc.tensor.transpose(otp[64:128, :], o[64:128, :],
                                            ident[64:128, 64:128],
                                            tile_position=(64, 64))
                        copy_out((nc.scalar, nc.vector)[t % 2],
                                 outT[p][:, qs], otp[:])

            # ---- output projection: y = outT.T @ pw (fp16 out) ----
            with tc.tile_pool(name="ypsp", bufs=4, space="PSUM") as ypsp:
                for tt in range(16):
                    ts_ = slice(tt * 128, (tt + 1) * 128)
                    for nt in range(2):
                        ns = slice(nt * 512, (nt + 1) * 512)
                        yp = ypsp.tile([128, 512], F32)
                        nc.tensor.matmul(yp[:], lhsT=outT[0][:, ts_],
                                         rhs=pwt[:, 0, ns],
                                         start=True, stop=False)
                        nc.tensor.matmul(yp[:], lhsT=outT[1][:, ts_],
                                         rhs=pwt[:, 1, ns],
                                         start=False, stop=True)
                        ys = wrk.tile([128, 512], F16, tag="ys")
                        copy_out((nc.vector, nc.scalar)[tt % 2], ys[:], yp[:])
                        nc.sync.dma_start(y_d[ts_, ns], ys[:])

    nc.finalize()
    return nc


def _host_prep(x, qkv_w, proj_w):
    """Per-core input maps + block selection (float64, matches fp32 ref)."""
    in_maps = []
    x64 = x.astype(np.float64)
    for core in range(NCORES):
        b = core // (NCORES // B)
        hg = core % (NCORES // B)
        heads = [hg * HPC + i for i in range(HPC)]

        xt = np.ascontiguousarray(x[b].T).astype(np.float16)

        wqkvT = np.empty((DIM, 768), np.float32)
        for p in range(2):
            hA, hB = heads[2 * p], heads[2 * p + 1]
            wqkvT[:, p*128:p*128+64] = qkv_w[hA*64:(hA+1)*64].T * SCALE
            wqkvT[:, p*128+64:p*128+128] = qkv_w[hB*64:(hB+1)*64].T * SCALE
            kbase = 256 + p * 256
            wqkvT[:, kbase:kbase+64] = qkv_w[DIM+hA*64:DIM+(hA+1)*64].T
            wqkvT[:, kbase+64:kbase+128] = qkv_w[DIM+hB*64:DIM+(hB+1)*64].T
            vbase = kbase + 128
            wqkvT[:, vbase:vbase+64] = qkv_w[2*DIM+hA*64:2*DIM+(hA+1)*64].T
            wqkvT[:, vbase+64:vbase+128] = qkv_w[2*DIM+hB*64:2*DIM+(hB+1)*64].T

        pw = np.ascontiguousarray(
            proj_w[:, heads[0]*64:(heads[-1]+1)*64].T).astype(np.float16)

        # float64 selection (matches fp32 reference ordering w/ margin)
        xb = x64[b].reshape(M, BS, DIM).mean(axis=1)
        selidx = np.zeros((1, 256), np.int32)
        wbias = np.zeros((128, 64), np.float32)
        for p in range(2):
            for hip in range(2):
                h = heads[2 * p + hip]
                qb_ = xb @ qkv_w[h*64:(h+1)*64].T.astype(np.float64)
                kb_ = xb @ qkv_w[DIM+h*64:DIM+(h+1)*64].T.astype(np.float64)
                c = qb_ @ kb_.T
                for i in range(M):
                    order = np.argsort(-c[i], kind="stable")
                    i1, i2 = int(order[0]), int(order[1])
                    col = p * 128 + i * 4 + hip * 2
                    selidx[0, col] = hip * N + i1 * 64
                    selidx[0, col + 1] = hip * N + i2 * 64
                    if i == i1 or i == i2:
                        wbias[hip*64:(hip+1)*64, p*32+i] = -30000.0
        in_maps.append({"xt": xt, "wq": wqkvT.astype(np.float16),
                        "pw": pw, "selidx": selidx, "wbias": wbias})
    return in_maps


def kernel(x, qkv_w, proj_w, proj_b):
    global _NC_CACHE, LAST_RESULTS
    x = np.asarray(x, np.float32)
    qkv_w = np.asarray(qkv_w, np.float32)
    proj_w = np.asarray(proj_w, np.float32)
    proj_b = np.asarray(proj_b, np.float32)

    if _NC_CACHE is None:
        _NC_CACHE = build_kernel()
    nc = _NC_CACHE

    in_maps = _host_prep(x, qkv_w, proj_w)
    res = run_bass_kernel_spmd(nc, in_maps, list(range(NCORES)))
    LAST_RESULTS = res

    out = np.zeros((B, N, DIM), np.float32)
    for core in range(NCORES):
        out[core // (NCORES // B)] += res.results[core]["y"].astype(np.float32)
    out += proj_b[None, None, :]
    return out


# revision 18
# speedup vs baseline: 1.3039x; 1.0000x over previous
"""Block-sparse attention (CABAttention) Trainium2 kernel, v2.

Sharding: 8 cores = 2 batches x 4 head-groups (2 pairs of heads each).
Per core, all in fp16 (fp32 PSUM/denominators):
 - qkv projection: fp16 matmuls over 8 K-chunks
 - top-2+diag block-sparse attention; block selection on host in
   float64 (matches fp32 reference ordering), consumed as dynamic
   access-pattern offsets on matmul moving operands (4 offsets per
   block loaded with one multi-register PE load)
 - softmax denominator via activation accum_out; normalization applied
   AFTER the AV matmul on [128,64] outputs (cheaper than normalizing
   [128,192] probabilities)
 - per-slot [64,64] PE transposes packed into disjoint quadrants
   (A row-group 0 / B row-group 64) so pairs run concurrently
 - head-B score/AV matmuls live natively in quadrant (0,64) with
   base-0 replicas (qkB = [qT-B | kkT-B], vdAB = [vd-A | vd-B]) so a
   single offset value serves both the scores and AV dynamic APs
 - output projection row-parallel: fp16 partial y per core, summed on
   host in fp32 (+bias).
"""
import sys

sys.path.insert(0, "/opt/trn_rl_repo")

import numpy as np

import concourse.bass as bass
import concourse.mybir as mybir
import concourse.tile as tile
from concourse import bacc
from concourse.bass import ds
from concourse.bass_utils import run_bass_kernel_spmd
from concourse.masks import make_identity

F32 = mybir.dt.float32
F16 = mybir.dt.float16
I32 = mybir.dt.int32

DIM = 1024
H = 16
HD = 64
BS = 64
N = 2048
B = 2
M = N // BS            # 32 blocks
SCALE = HD ** -0.5
NCORES = 8
HPC = H // (NCORES // B)   # 4 heads per core

_NC_CACHE = None
LAST_RESULTS = None


def build_kernel(stage=5, sub=3):
    nc = bacc.Bacc(None)
    xt_d = nc.dram_tensor("xt", [DIM, N], F16, kind="ExternalInput")
    wq_d = nc.dram_tensor("wq", [DIM, 768], F16, kind="ExternalInput")
    pw_d = nc.dram_tensor("pw", [256, DIM], F16, kind="ExternalInput")
    idx_d = nc.dram_tensor("selidx", [1, 256], I32, kind="ExternalInput")
    wb_d = nc.dram_tensor("wbias", [128, 64], F32, kind="ExternalInput")
    y_d = nc.dram_tensor("y", [N, DIM], F16, kind="ExternalOutput")

    EXP = mybir.ActivationFunctionType.Exp
    ADD = mybir.AluOpType.add
    MUL = mybir.AluOpType.mult

    with tile.TileContext(nc) as tc:
        with tc.tile_pool(name="big", bufs=1) as big, \
             tc.tile_pool(name="wrk", bufs=4) as wrk:

            # ---- persistent SBUF tensors ----
            xts = big.tile([128, 8, N], F16)
            wqs = big.tile([128, 8, 768], F16)
            pwt = big.tile([128, 2, DIM], F16)
            idx = big.tile([1, 256], I32)
            wb = big.tile([128, 64], F32)
            qT = [big.tile([128, N], F16, name=f"qT{i}") for i in range(2)]
            kkT = [big.tile([128, N], F16, name=f"kkT{i}") for i in range(2)]
            vvT = [big.tile([128, N], F16, name=f"vvT{i}") for i in range(2)]
            # base-0 replicas for head B: [qT-B | kkT-B] so one dynamic
            # offset value (N + 64j) indexes the kk region
            qkB = [big.tile([64, 2 * N], F16, name=f"qkB{i}") for i in range(2)]
            # [vd-A | vd-B] keys-major v, same offset convention
            vdAB = [big.tile([64, 2 * N], F16, name=f"vdAB{i}") for i in range(2)]
            outT = [big.tile([128, N], F16, name=f"outT{i}") for i in range(2)]
            identf = big.tile([128, 128], F32)
            ident = big.tile([128, 128], F16)

            # ---- input DMAs (split for pipelining) ----
            xt_v = xt_d[:].rearrange("(a p) n -> p a n", p=128)
            wq_v = wq_d[:].rearrange("(a p) n -> p a n", p=128)
            pw_v = pw_d[:].rearrange("(a p) n -> p a n", p=128)
            for k in range(8):
                nc.sync.dma_start(xts[:, k, :], xt_v[:, k, :])
                nc.sync.dma_start(wqs[:, k, :], wq_v[:, k, :])
            nc.sync.dma_start(pwt[:], pw_v[:])
            nc.sync.dma_start(idx[:], idx_d[:])
            nc.sync.dma_start(wb[:], wb_d[:])

            make_identity(nc, identf[:])
            nc.vector.tensor_copy(ident[:], identf[:])
            if stage < 5:
                for t_ in outT:
                    nc.gpsimd.memset(t_[:], 0.0)

            def copy_out(eng, dst, src):
                if eng is nc.scalar:
                    eng.copy(dst, src)
                else:
                    eng.tensor_copy(dst, src)

            # ---- qkv projection: fp16, 8 K-chunk accumulation ----
            # mt -> target; pair-0 tiles first so attention p0 can start
            tgt = {0: qT[0], 1: qT[1], 2: kkT[0], 3: vvT[0],
                   4: kkT[1], 5: vvT[1]}
            ci = 0
            with tc.tile_pool(name="qkps", bufs=4, space="PSUM") as qkps, \
                 tc.tile_pool(name="vtps", bufs=2, space="PSUM") as vtps:

                def emit_qkv(mts):
                    nonlocal ci
                    for mt in mts:
                        for nt in range(4):
                            ps = qkps.tile([128, 512], F32)
                            for k in range(8):
                                nc.tensor.matmul(
                                    ps[:],
                                    lhsT=wqs[:, k, mt * 128:(mt + 1) * 128],
                                    rhs=xts[:, k, nt * 512:(nt + 1) * 512],
                                    start=(k == 0), stop=(k == 7))
                            eng = (nc.vector, nc.scalar)[ci % 2]
                            ci += 1
                            copy_out(eng, tgt[mt][:, nt * 512:(nt + 1) * 512],
                                     ps[:])

                def emit_vdup(p):
                    nonlocal ci
                    for j in range(M):
                        tp = vtps.tile([64, 128], F16)
                        nc.tensor.transpose(
                            tp[:], vvT[p][:, j * 64:(j + 1) * 64], ident[:])
                        # one strided copy: A block -> cols [j*64], B block
                        # -> cols [N + j*64] of vdAB
                        dst = vdAB[p][:].rearrange(
                            "q (g c) -> q g c", g=2)[:, :, j * 64:(j + 1) * 64]
                        src = tp[:].rearrange("q (g c) -> q g c", g=2)
                        eng = (nc.vector, nc.scalar)[ci % 2]
                        ci += 1
                        copy_out(eng, dst, src)

                emit_qkv((0, 2, 3))
                nc.sync.dma_start(qkB[0][:, 0:N], qT[0][64:128, :])
                nc.sync.dma_start(qkB[0][:, N:2 * N], kkT[0][64:128, :])
                emit_vdup(0)
                emit_qkv((1, 4, 5))
                nc.sync.dma_start(qkB[1][:, 0:N], qT[1][64:128, :])
                nc.sync.dma_start(qkB[1][:, N:2 * N], kkT[1][64:128, :])
                emit_vdup(1)

            # ---- block-sparse attention ----
            with tc.tile_pool(name="spsp", bufs=2, space="PSUM") as spsp, \
                 tc.tile_pool(name="ptps", bufs=2, space="PSUM") as ptps, \
                 tc.tile_pool(name="avps", bufs=2, space="PSUM") as avps, \
                 tc.tile_pool(name="otps", bufs=2, space="PSUM") as otps:
                anchors = []
                for p in (range(2) if stage >= 2 else []):
                    for qb in range(M):
                        t = p * M + qb
                        base = p * 128 + qb * 4
                        col = p * 32 + qb
                        qs = slice(qb * 64, (qb + 1) * 64)

                        # 4 dynamic offsets, one multi-register PE load
                        eng = nc.tensor
                        regs = [eng.alloc_register(f"off_{base + c}")
                                for c in range(4)]
                        li = eng.reg_load(regs, idx[0:1, base:base + 4])
                        if t >= 3:
                            tile.add_dep_helper(
                                li.ins, anchors[t - 3].ins, sync=False,
                                reason="bound PE register live range")
                        offs = []
                        for c in range(4):
                            lo = 0 if c < 2 else N
                            offs.append(eng.snap(
                                regs[c], donate=True, min_val=lo,
                                max_val=lo + N - 64))
                        soA = [offs[0], offs[1], qb * 64]
                        soB = [offs[2], offs[3], N + qb * 64]

                        # scores^T quadrant-packed: A rows 0-63 via
                        # (0,0), B rows 64-127 via (0,64)
                        sps = spsp.tile([128, 192], F32)
                        for s in range(2):
                            cs = slice(s * 64, (s + 1) * 64)
                            nc.tensor.matmul(
                                sps[0:64, cs], lhsT=qT[p][0:64, qs],
                                rhs=kkT[p][0:64, ds(soA[s], 64)],
                                start=True, stop=True)
                            nc.tensor.matmul(
                                sps[64:128, cs], lhsT=qkB[p][:, qs],
                                rhs=qkB[p][:, ds(soB[s], 64)],
                                start=True, stop=True,
                                skip_group_check=True,
                                tile_position=(0, 64))
                        # diag slot (static offsets)
                        nc.tensor.matmul(
                            sps[0:64, 128:192], lhsT=qT[p][0:64, qs],
                            rhs=kkT[p][0:64, ds(soA[2], 64)],
                            start=True, stop=True)
                        nc.tensor.matmul(
                            sps[64:128, 128:192], lhsT=qkB[p][:, qs],
                            rhs=qkB[p][:, ds(soB[2], 64)],
                            start=True, stop=True,
                            skip_group_check=True,
                            tile_position=(0, 64))

                        # duplicate-diag mask, then one exp with accum den
                        nc.vector.tensor_scalar(
                            sps[:, 128:192], sps[:, 128:192],
                            wb[:, col:col + 1], None, op0=ADD)
                        pu = wrk.tile([128, 192], F16, tag="pu")
                        den = wrk.tile([128, 1], F32, tag="den")
                        nc.scalar.activation(pu[:], sps[:], EXP,
                                             accum_out=den[:])

                        if stage < 3:
                            anchors.append(li)
                            continue
                        # transpose probs per (head, slot): [64,64] pairs
                        # in disjoint quadrants run concurrently
                        # full-height transpose per slot: [128 q, 64 k]
                        # -> [64 k, 128 (qA|qB)], both heads at base 0
                        pt = ptps.tile([64, 384], F16)
                        for s in range(3):
                            cs = slice(s * 64, (s + 1) * 64)
                            nc.tensor.transpose(
                                pt[0:64, s * 128:(s + 1) * 128],
                                pu[:, cs], ident[:])
                        pts = wrk.tile([64, 384], F16, tag="pts")
                        nc.vector.tensor_copy(pts[:], pt[:])

                        if stage < 4:
                            anchors.append(li)
                            continue
                        # AV (unnormalized): A -> av[0:64], B -> av[64:128]
                        av = avps.tile([128, 64], F32)
                        for s in range(3):
                            nc.tensor.matmul(
                                av[0:64, :],
                                lhsT=pts[0:64, s * 128:s * 128 + 64],
                                rhs=vdAB[p][:, ds(soA[s], 64)],
                                start=(s == 0), stop=(s == 2))
                            mi = nc.tensor.matmul(
                                av[64:128, :],
                                lhsT=pts[0:64, s * 128 + 64:(s + 1) * 128],
                                rhs=vdAB[p][:, ds(soB[s], 64)],
                                start=(s == 0), stop=(s == 2),
                                skip_group_check=True,
                                tile_position=(0, 64))
                        anchors.append(mi)

                        # normalize post-AV (64 cols instead of 192)
                        rden = wrk.tile([128, 1], F32, tag="rden")
                        nc.vector.reciprocal(rden[:], den[:])
                        o = wrk.tile([128, 64], F16, tag="o")
                        nc.vector.tensor_scalar(o[:], av[:], rden[:, 0:1],
                                                None, op0=MUL)

                        if stage < 5:
                            continue
                        # transpose back to [hd, q]; A/B quadrant pair
                        otp = otps.tile([128, 64], F16)
                        nc.tensor.transpose(otp[0:64, :], o[0:64, :],
                                            ident[0:64, 0:64])
                        nc.tensor.transpose(otp[64:128, :], o[64:128, :],
                                            ident[64:128, 64:128],
                                            tile_position=(64, 64))
                        nc.scalar.copy(outT[p][:, qs], otp[:])

            # ---- output projection: y = outT.T @ pw (fp16 out) ----
            with tc.tile_pool(name="ypsp", bufs=4, space="PSUM") as ypsp:
                for tt in range(16):
                    ts_ = slice(tt * 128, (tt + 1) * 128)
                    for nt in range(2):
                        ns = slice(nt * 512, (nt + 1) * 512)
                        yp = ypsp.tile([128, 512], F32)
                        nc.tensor.matmul(yp[:], lhsT=outT[0][:, ts_],
                                         rhs=pwt[:, 0, ns],
                                         start=True, stop=False)
                        nc.tensor.matmul(yp[:], lhsT=outT[1][:, ts_],
                                         rhs=pwt[:, 1, ns],
                                         start=False, stop=True)
                        ys = wrk.tile([128, 512], F16, tag="ys")
                        copy_out((nc.vector, nc.scalar)[tt % 2], ys[:], yp[:])
                        nc.sync.dma_start(y_d[ts_, ns], ys[:])

    nc.finalize()
    return nc


def _host_prep(x, qkv_w, proj_w):
    """Per-core input maps + block selection (float64, matches fp32 ref)."""
    in_maps = []
    x64 = x.astype(np.float64)
    for core in range(NCORES):
        b = core // (NCORES // B)
        hg = core % (NCORES // B)
        heads = [hg * HPC + i for i in range(HPC)]

        xt = np.ascontiguousarray(x[b].T).astype(np.float16)

        wqkvT = np.empty((DIM, 768), np.float32)
        for p in range(2):
            hA, hB = heads[2 * p], heads[2 * p + 1]
            wqkvT[:, p*128:p*128+64] = qkv_w[hA*64:(hA+1)*64].T * SCALE
            wqkvT[:, p*128+64:p*128+128] = qkv_w[hB*64:(hB+1)*64].T * SCALE
            kbase = 256 + p * 256
            wqkvT[:, kbase:kbase+64] = qkv_w[DIM+hA*64:DIM+(hA+1)*64].T
            wqkvT[:, kbase+64:kbase+128] = qkv_w[DIM+hB*64:DIM+(hB+1)*64].T
            vbase = kbase + 128
            wqkvT[:, vbase:vbase+64] = qkv_w[2*DIM+hA*64:2*DIM+(hA+1)*64].T
            wqkvT[:, vbase+64:vbase+128] = qkv_w[2*DIM+hB*64:2*DIM+(hB+1)*64].T

        pw = np.ascontiguousarray(
            proj_w[:, heads[0]*64:(heads[-1]+1)*64].T).astype(np.float16)

        # float64 selection (matches fp32 reference ordering w/ margin)
        xb = x64[b].reshape(M, BS, DIM).mean(axis=1)
        selidx = np.zeros((1, 256), np.int32)
        wbias = np.zeros((128, 64), np.float32)
        for p in range(2):
            for hip in range(2):
                h = heads[2 * p + hip]
                qb_ = xb @ qkv_w[h*64:(h+1)*64].T.astype(np.float64)
                kb_ = xb @ qkv_w[DIM+h*64:DIM+(h+1)*64].T.astype(np.float64)
                c = qb_ @ kb_.T
                for i in range(M):
                    order = np.argsort(-c[i], kind="stable")
                    i1, i2 = int(order[0]), int(order[1])
                    col = p * 128 + i * 4 + hip * 2
                    selidx[0, col] = hip * N + i1 * 64
                    selidx[0, col + 1] = hip * N + i2 * 64
                    if i == i1 or i == i2:
                        wbias[hip*64:(hip+1)*64, p*32+i] = -30000.0
        in_maps.append({"xt": xt, "wq": wqkvT.astype(np.float16),
                        "pw": pw, "selidx": selidx, "wbias": wbias})
    return in_maps


def kernel(x, qkv_w, proj_w, proj_b):
    global _NC_CACHE, LAST_RESULTS
    x = np.asarray(x, np.float32)
    qkv_w = np.asarray(qkv_w, np.float32)
    proj_w = np.asarray(proj_w, np.float32)
    proj_b = np.asarray(proj_b, np.float32)

    if _NC_CACHE is None:
        _NC_CACHE = build_kernel()
    nc = _NC_CACHE

    in_maps = _host_prep(x, qkv_w, proj_w)
    res = run_bass_kernel_spmd(nc, in_maps, list(range(NCORES)))
    LAST_RESULTS = res

    out = np.zeros((B, N, DIM), np.float32)
    for core in range(NCORES):
        out[core // (NCORES // B)] += res.results[core]["y"].astype(np.float32)
    out += proj_b[None, None, :]
    return out
